# revision 10
# baseline (speedup 1.0000x reference)
"""CNN char encoder (conv widths 1/2/3 -> tanh -> max over time -> highway)
as a Bass/Tile kernel for 8 Trainium2 NeuronCores.

Sharding: data-parallel over the 4096 = 32*128 flattened words; 512 words per
core; all weights replicated. Everything on-chip is feature-major
([feature_partition, word] tiles); the host does the final transpose back to
(B, S, OUT_DIM).

Input is shipped in a bf16 "pair" layout
    xp[r, n*10 + u] = x[word n, char position 2u+d, channel c]
with r = c for d=0 and r = 64+c for d=1 (rows 50..63 zero padding: matmul
operand base partitions must be 0/32/64 and must match between lhsT and rhs;
a [base64 K=50, base0 K=50] accumulation group also crashes the device, so
odd-position first matmuls are widened to zero-padded K=114 at base 0).
A width-3 conv output position needs 2 matmuls (K=114 + K=50); width-1/2
convs read row-blocks of the same tile. All matmuls are bf16 (fp32 matmuls
cost a serialized ~284ns weight load + 2-pass streaming; bf16 streams 1
column/cycle with the weight load hidden).

Conv positions are evaluated two-at-a-time into a 2-bank PSUM tile
[128, 2, 512] to halve per-op overheads downstream. Max over time is
tanh-first (tanh is monotone; conv bias folded into the ACT op), split
between two routes to balance ACT vs DVE load:
  - ACT route: tanh PSUM -> bf16 [128,1024] scratch; DVE bf16 max into a
    1024-wide running max (2x mode)
  - DVE route: fp32 max straight from PSUM into a 1024-wide accumulator,
    single tanh at the end
and the two 512-halves are max-combined once per bank at the end. The
highway h/t branches share one 2-bank PSUM tile per output block; the
epilogue t*(h-f)+f runs on DVE in bf16 with the final add in fp32.
"""

import numpy as np
import ml_dtypes

import concourse.bass as bass
import concourse.tile as tile
from concourse import bacc, mybir
from concourse.bass_utils import run_bass_kernel_spmd

F32 = mybir.dt.float32
BF16 = mybir.dt.float16  # fp16: same PE/DVE speed class as bf16, 8x finer mantissa
ACTF = mybir.ActivationFunctionType

N_CORES = 8
B, S, L, C = 32, 128, 20, 50
NW = B * S               # 4096 words total
WPC = NW // N_CORES      # 512 words per core
U = L // 2               # 10 pairs per word
D1 = 64                  # partition base of the d=1 block
R = D1 + C               # 114 rows in the pair layout
OUT_DIM = 768


def _routes(n, n_act):
    """Spread n_act ACT-routed items evenly through a bank."""
    if n_act <= 0:
        return [False] * n
    step = n / n_act
    picks = {int(i * step) for i in range(n_act)}
    return [i in picks for i in range(n)]


# tuning knobs: per bank, how many POSITION PAIRS go down the ACT route
# (the rest take the fp32-from-PSUM DVE route).
ACT_PAIRS = {"w1": 7, "w2": 7, "w3_0": 6, "w3_1": 6, "w3_2": 6, "w3_3": 6}


def build_nc():
    nc = bacc.Bacc(
        "TRN2", target_bir_lowering=False, debug=False, num_devices=N_CORES
    )

    xp = nc.dram_tensor("xp", [R, WPC * U], BF16, kind="ExternalInput")
    w1p = nc.dram_tensor("w1p", [R, 128], BF16, kind="ExternalInput")
    w2p = nc.dram_tensor("w2p", [R, 128], BF16, kind="ExternalInput")
    w2o = nc.dram_tensor("w2o", [R, 128], BF16, kind="ExternalInput")
    w2z = nc.dram_tensor("w2z", [R, 128], BF16, kind="ExternalInput")
    w3a = nc.dram_tensor("w3a", [R, 512], BF16, kind="ExternalInput")
    w3t2 = nc.dram_tensor("w3t2", [C, 512], BF16, kind="ExternalInput")
    w3o = nc.dram_tensor("w3o", [R, 512], BF16, kind="ExternalInput")
    w3d = nc.dram_tensor("w3d", [R, 512], BF16, kind="ExternalInput")
    biasp = nc.dram_tensor("biasp", [128, 18], F32, kind="ExternalInput")
    whp = nc.dram_tensor("whp", [128, 36 * 128], BF16, kind="ExternalInput")
    wtp = nc.dram_tensor("wtp", [128, 36 * 128], BF16, kind="ExternalInput")
    out = nc.dram_tensor("out", [OUT_DIM, WPC], F32, kind="ExternalOutput")

    with tile.TileContext(nc) as tc:
        with (
            tc.tile_pool(name="singles", bufs=1) as singles,
            tc.tile_pool(name="psum", bufs=3, space="PSUM") as psum,
            tc.tile_pool(name="gscr", bufs=4) as gscr,
            tc.tile_pool(name="hwtiles", bufs=2) as hwt,
        ):
            sb_x = singles.tile([R, WPC * U], BF16)
            nc.sync.dma_start(out=sb_x, in_=xp.ap())
            sb_w1 = singles.tile([R, 128], BF16)
            nc.sync.dma_start(out=sb_w1, in_=w1p.ap())
            sb_w2 = singles.tile([R, 128], BF16)
            nc.sync.dma_start(out=sb_w2, in_=w2p.ap())
            sb_w2o = singles.tile([R, 128], BF16)
            nc.sync.dma_start(out=sb_w2o, in_=w2o.ap())
            sb_w2z = singles.tile([R, 128], BF16)
            nc.sync.dma_start(out=sb_w2z, in_=w2z.ap())
            sb_bias = singles.tile([128, 18], F32)
            nc.sync.dma_start(out=sb_bias, in_=biasp.ap())
            sb_w3a = singles.tile([R, 512], BF16)
            nc.sync.dma_start(out=sb_w3a, in_=w3a.ap())
            sb_w3t2 = singles.tile([C, 512], BF16)
            nc.sync.dma_start(out=sb_w3t2, in_=w3t2.ap())
            sb_w3o = singles.tile([R, 512], BF16)
            nc.sync.dma_start(out=sb_w3o, in_=w3o.ap())
            sb_w3d = singles.tile([R, 512], BF16)
            nc.sync.dma_start(out=sb_w3d, in_=w3d.ap())
            sb_wh = singles.tile([128, 36 * 128], BF16)
            nc.sync.dma_start(out=sb_wh, in_=whp.ap())
            sb_wt = singles.tile([128, 36 * 128], BF16)
            nc.sync.dma_start(out=sb_wt, in_=wtp.ap())

            # [R, words, pairs] view for strided column access
            xv = sb_x.rearrange("p (n u) -> p n u", u=U)

            def xcols(row0, nrows, u):
                return xv[row0 : row0 + nrows, :, u]

            feat = []
            for j in range(6):
                fj = singles.tile([128, WPC], BF16, name=f"feat{j}")
                feat.append(fj)

            def conv_bank(name, feat_tile, bias_col, positions, n_act):
                """positions: list (per t) of lists of (lhsT_ap, rhs_ap).
                Processes positions in pairs sharing a 2-bank PSUM tile."""
                n = len(positions)
                pairs = [(i, i + 1 if i + 1 < n else None)
                         for i in range(0, n, 2)]
                # route full pairs; a trailing single always goes DVE
                full = [p for p in pairs if p[1] is not None]
                routes = dict(zip(full, _routes(len(full), n_act)))
                featw = None  # bf16 [128, 2*WPC] running max of tanh (A route)
                accw = None   # f32 [128, 2*WPC] running max pre-tanh (D route)
                featw_started = accw_started = False
                accw_wide = False
                for pr in pairs:
                    i0, i1 = pr
                    width = 2 if i1 is not None else 1
                    y2 = psum.tile([128, 2, WPC], F32, name="ypsum2", bufs=3)
                    for sl, ipos in enumerate((i0, i1)[:width]):
                        mms = positions[ipos]
                        for i, (lhsT, rhs) in enumerate(mms):
                            nc.tensor.matmul(
                                y2[:, sl, :], lhsT, rhs,
                                start=(i == 0), stop=(i == len(mms) - 1),
                            )
                    yflat = y2.rearrange("p a b -> p (a b)")[:, : width * WPC]
                    if routes.get(pr, False):
                        if not featw_started:
                            featw = singles.tile(
                                [128, 2 * WPC], BF16, name=f"fw_{name}"
                            )
                            nc.scalar.activation(
                                featw, yflat, ACTF.Tanh, bias=bias_col
                            )
                            featw_started = True
                        else:
                            scr = gscr.tile(
                                [128, 2 * WPC], BF16, name="gscr_t", bufs=4
                            )
                            nc.scalar.activation(
                                scr, yflat, ACTF.Tanh, bias=bias_col
                            )
                            nc.vector.tensor_max(featw, featw, scr)
                    else:
                        if not accw_started:
                            accw = singles.tile(
                                [128, 2 * WPC], F32, name=f"aw_{name}"
                            )
                            nc.vector.tensor_copy(accw[:, : width * WPC], yflat)
                            accw_started = True
                            accw_wide = width == 2
                        else:
                            nc.vector.tensor_max(
                                accw[:, : width * WPC],
                                accw[:, : width * WPC],
                                yflat,
                            )
                            accw_wide = accw_wide or width == 2
                # fold the two halves + merge routes into feat_tile
                amax = None  # f32 [128, WPC] pre-tanh max of the D route
                if accw_started:
                    amax = gscr.tile([128, WPC], F32, name="amax", bufs=2)
                    if accw_wide:
                        nc.vector.tensor_max(
                            amax, accw[:, 0:WPC], accw[:, WPC : 2 * WPC]
                        )
                    else:
                        nc.vector.tensor_copy(amax, accw[:, 0:WPC])
                if featw_started and accw_started:
                    nc.vector.tensor_max(
                        feat_tile, featw[:, 0:WPC], featw[:, WPC : 2 * WPC]
                    )
                    mrg = gscr.tile([128, WPC], BF16, name="mrg", bufs=2)
                    nc.scalar.activation(mrg, amax, ACTF.Tanh, bias=bias_col)
                    nc.vector.tensor_max(feat_tile, feat_tile, mrg)
                elif featw_started:
                    nc.vector.tensor_max(
                        feat_tile, featw[:, 0:WPC], featw[:, WPC : 2 * WPC]
                    )
                else:
                    nc.scalar.activation(
                        feat_tile, amax, ACTF.Tanh, bias=bias_col
                    )

            # width-1 bank -> feat[0]; position t = 2u+d reads row-block d
            pos = []
            for t in range(20):
                d, u = t % 2, t // 2
                r0 = 0 if d == 0 else D1
                pos.append([(sb_w1[r0 : r0 + C, :], xcols(r0, C, u))])
            conv_bank("w1", feat[0], sb_bias[:, 0:1], pos, ACT_PAIRS["w1"])

            # width-2 bank -> feat[1]
            pos = []
            for t in range(19):
                if t % 2 == 0:
                    u = t // 2
                    pos.append([(sb_w2[0:R, :], xcols(0, R, u))])
                else:
                    u = (t - 1) // 2
                    pos.append(
                        [
                            (sb_w2z[0:R, :], xcols(0, R, u)),
                            (sb_w2o[0:C, :], xcols(0, C, u + 1)),
                        ]
                    )
            conv_bank("w2", feat[1], sb_bias[:, 1:2], pos, ACT_PAIRS["w2"])

            # width-3 banks -> feat[2..5]
            for j in range(4):
                cs = slice(j * 128, (j + 1) * 128)
                aj = sb_w3a[:, cs]
                t2j = sb_w3t2[:, cs]
                oj = sb_w3o[:, cs]
                dj = sb_w3d[:, cs]
                pos = []
                for t in range(18):
                    if t % 2 == 0:
                        u = t // 2
                        pos.append(
                            [
                                (aj[0:R, :], xcols(0, R, u)),
                                (t2j[0:C, :], xcols(0, C, u + 1)),
                            ]
                        )
                    else:
                        u = (t - 1) // 2
                        pos.append(
                            [
                                (oj[0:R, :], xcols(0, R, u)),
                                (dj[0:R, :], xcols(0, R, u + 1)),
                            ]
                        )
                conv_bank(
                    f"w3_{j}",
                    feat[2 + j],
                    sb_bias[:, 2 + j : 3 + j],
                    pos,
                    ACT_PAIRS[f"w3_{j}"],
                )

            # highway: h = relu(Wh f + bh), t = sig(Wt f + bt),
            # out = t*(h-f) + f, all feature-major [128 out-feats, 512 words]
            for ot in range(6):
                hp2 = psum.tile([128, 2, WPC], F32, name="ypsum2", bufs=3)
                for br, wsb in ((0, sb_wh), (1, sb_wt)):
                    for kt in range(6):
                        blk = (ot * 6 + kt) * 128
                        nc.tensor.matmul(
                            hp2[:, br, :],
                            wsb[:, blk : blk + 128],
                            feat[kt],
                            start=(kt == 0),
                            stop=(kt == 5),
                        )
                h_sb = hwt.tile([128, WPC], BF16, name="h_sb", bufs=2)
                nc.scalar.activation(
                    h_sb, hp2[:, 0, :], ACTF.Relu,
                    bias=sb_bias[:, 6 + ot : 7 + ot],
                )
                t_sb = hwt.tile([128, WPC], BF16, name="t_sb", bufs=2)
                nc.scalar.activation(
                    t_sb, hp2[:, 1, :], ACTF.Sigmoid,
                    bias=sb_bias[:, 12 + ot : 13 + ot],
                )
                out_sb = hwt.tile([128, WPC], F32, name="out_sb", bufs=2)
                nc.vector.tensor_sub(h_sb, h_sb, feat[ot])
                nc.vector.tensor_mul(h_sb, t_sb, h_sb)
                nc.vector.tensor_add(out_sb, h_sb, feat[ot])
                nc.sync.dma_start(
                    out=out.ap()[ot * 128 : (ot + 1) * 128, :], in_=out_sb
                )

    nc.compile()
    return nc


def pack_inputs(ts10_input, conv_w0, conv_b0, conv_w1, conv_b1, conv_w2,
                conv_b2, wh_w, wh_b, wt_w, wt_b):
    f = np.float32
    bf = np.float16

    def padded(top, bottom, ncols):
        arr = np.zeros((R, ncols), f)
        if top is not None:
            arr[0:C] = top
        if bottom is not None:
            arr[D1:R] = bottom
        return arr.astype(bf)

    X = np.ascontiguousarray(ts10_input, dtype=f).reshape(NW, L, C)
    w1t = conv_w0[:, :, 0].T
    w1p = padded(w1t, w1t, 128)
    w2p = padded(conv_w1[:, :, 0].T, conv_w1[:, :, 1].T, 128)
    w2o = padded(conv_w1[:, :, 1].T, conv_w1[:, :, 0].T, 128)
    w2z = padded(None, conv_w1[:, :, 0].T, 128)
    w3a = padded(conv_w2[:, :, 0].T, conv_w2[:, :, 1].T, 512)
    w3t2 = np.ascontiguousarray(conv_w2[:, :, 2].T).astype(bf)
    w3o = padded(None, conv_w2[:, :, 0].T, 512)
    w3d = padded(conv_w2[:, :, 1].T, conv_w2[:, :, 2].T, 512)
    biasp = np.zeros((128, 18), f)
    biasp[:, 0] = conv_b0
    biasp[:, 1] = conv_b1
    for j in range(4):
        biasp[:, 2 + j] = conv_b2[j * 128 : (j + 1) * 128]
    for ot in range(6):
        biasp[:, 6 + ot] = wh_b[ot * 128 : (ot + 1) * 128]
        biasp[:, 12 + ot] = wt_b[ot * 128 : (ot + 1) * 128]
    whp = np.ascontiguousarray(
        wh_w.reshape(6, 128, 6, 128).transpose(3, 0, 2, 1).reshape(128, 36 * 128)
    ).astype(bf)
    wtp = np.ascontiguousarray(
        wt_w.reshape(6, 128, 6, 128).transpose(3, 0, 2, 1).reshape(128, 36 * 128)
    ).astype(bf)
    shared = dict(w1p=w1p, w2p=w2p, w2o=w2o, w2z=w2z, w3a=w3a, w3t2=w3t2,
                  w3o=w3o, w3d=w3d, biasp=biasp, whp=whp, wtp=wtp)
    in_maps = []
    for c in range(N_CORES):
        Xc = X[c * WPC : (c + 1) * WPC]            # [512, 20, 50]
        pair = Xc.reshape(WPC, U, 2, C).transpose(2, 3, 0, 1)  # [2, C, 512, U]
        xpc = np.zeros((R, WPC * U), f)
        xpc[0:C] = pair[0].reshape(C, WPC * U)
        xpc[D1:R] = pair[1].reshape(C, WPC * U)
        in_maps.append(dict(xp=xpc.astype(bf), **shared))
    return in_maps


_NC_CACHE = None


def get_nc():
    global _NC_CACHE
    if _NC_CACHE is None:
        _NC_CACHE = build_nc()
    return _NC_CACHE


def kernel(**inputs):
    in_maps = pack_inputs(**{k: np.asarray(v) for k, v in inputs.items()})
    nc = get_nc()
    res = run_bass_kernel_spmd(nc, in_maps, core_ids=list(range(N_CORES)))
    full = np.empty((NW, OUT_DIM), np.float32)
    for c in range(N_CORES):
        full[c * WPC : (c + 1) * WPC] = res.results[c]["out"].T
    return full.reshape(B, S, OUT_DIM)


# revision 11
# speedup vs baseline: 1.6176x; 1.6176x over previous
"""CNN char encoder (conv widths 1/2/3 -> tanh -> max over time -> highway)
as a Bass/Tile kernel for 8 Trainium2 NeuronCores.

Sharding: data-parallel over the 4096 = 32*128 flattened words; 512 words per
core; all weights replicated. Everything on-chip is feature-major
([feature_partition, word] tiles); the host does the final transpose back to
(B, S, OUT_DIM).

Input is shipped in a bf16 "pair" layout
    xp[r, n*10 + u] = x[word n, char position 2u+d, channel c]
with r = c for d=0 and r = 64+c for d=1 (rows 50..63 zero padding: matmul
operand base partitions must be 0/32/64 and must match between lhsT and rhs;
a [base64 K=50, base0 K=50] accumulation group also crashes the device, so
odd-position first matmuls are widened to zero-padded K=114 at base 0).
A width-3 conv output position needs 2 matmuls (K=114 + K=50); width-1/2
convs read row-blocks of the same tile. All matmuls are bf16 (fp32 matmuls
cost a serialized ~284ns weight load + 2-pass streaming; bf16 streams 1
column/cycle with the weight load hidden).

Conv positions are evaluated two-at-a-time into a 2-bank PSUM tile
[128, 2, 512] to halve per-op overheads downstream. Max over time is
tanh-first (tanh is monotone; conv bias folded into the ACT op), split
between two routes to balance ACT vs DVE load:
  - ACT route: tanh PSUM -> bf16 [128,1024] scratch; DVE bf16 max into a
    1024-wide running max (2x mode)
  - DVE route: fp32 max straight from PSUM into a 1024-wide accumulator,
    single tanh at the end
and the two 512-halves are max-combined once per bank at the end. The
highway h/t branches share one 2-bank PSUM tile per output block; the
epilogue t*(h-f)+f runs on DVE in bf16 with the final add in fp32.
"""

import numpy as np
import ml_dtypes

import concourse.bass as bass
import concourse.tile as tile
from concourse import bacc, mybir
from concourse.bass_utils import run_bass_kernel_spmd

F32 = mybir.dt.float32
BF16 = mybir.dt.float16  # fp16: same PE/DVE speed class as bf16, 8x finer mantissa
ACTF = mybir.ActivationFunctionType

N_CORES = 8
B, S, L, C = 32, 128, 20, 50
NW = B * S               # 4096 words total
WPC = NW // N_CORES      # 512 words per core
U = L // 2               # 10 pairs per word
D1 = 64                  # partition base of the d=1 block
R = D1 + C               # 114 rows in the pair layout
OUT_DIM = 768


def _routes(n, n_act):
    """Spread n_act ACT-routed items evenly through a bank."""
    if n_act <= 0:
        return [False] * n
    step = n / n_act
    picks = {int(i * step) for i in range(n_act)}
    return [i in picks for i in range(n)]


# tuning knobs: per bank, how many POSITION PAIRS go down the ACT route
# (the rest take the fp32-from-PSUM DVE route).
ACT_PAIRS = {"w1": 7, "w2": 7, "w3_0": 6, "w3_1": 6, "w3_2": 6, "w3_3": 6}


def build_nc():
    nc = bacc.Bacc(
        "TRN2", target_bir_lowering=False, debug=False, num_devices=N_CORES
    )

    xp = nc.dram_tensor("xp", [R, WPC * U], BF16, kind="ExternalInput")
    w1p = nc.dram_tensor("w1p", [R, 128], BF16, kind="ExternalInput")
    w2p = nc.dram_tensor("w2p", [R, 128], BF16, kind="ExternalInput")
    w2o = nc.dram_tensor("w2o", [R, 128], BF16, kind="ExternalInput")
    w2z = nc.dram_tensor("w2z", [R, 128], BF16, kind="ExternalInput")
    w3a = nc.dram_tensor("w3a", [R, 512], BF16, kind="ExternalInput")
    w3t2 = nc.dram_tensor("w3t2", [C, 512], BF16, kind="ExternalInput")
    w3o = nc.dram_tensor("w3o", [R, 512], BF16, kind="ExternalInput")
    w3d = nc.dram_tensor("w3d", [R, 512], BF16, kind="ExternalInput")
    biasp = nc.dram_tensor("biasp", [128, 18], F32, kind="ExternalInput")
    whp = nc.dram_tensor("whp", [128, 36 * 128], BF16, kind="ExternalInput")
    wtp = nc.dram_tensor("wtp", [128, 36 * 128], BF16, kind="ExternalInput")
    out = nc.dram_tensor("out", [OUT_DIM, WPC], F32, kind="ExternalOutput")

    with tile.TileContext(nc) as tc:
        with (
            tc.tile_pool(name="singles", bufs=1) as singles,
            tc.tile_pool(name="psum", bufs=3, space="PSUM") as psum,
            tc.tile_pool(name="gscr", bufs=4) as gscr,
            tc.tile_pool(name="hwtiles", bufs=2) as hwt,
        ):
            sb_x = singles.tile([R, WPC * U], BF16)
            nc.sync.dma_start(out=sb_x, in_=xp.ap())
            sb_w1 = singles.tile([R, 128], BF16)
            nc.sync.dma_start(out=sb_w1, in_=w1p.ap())
            sb_w2 = singles.tile([R, 128], BF16)
            nc.sync.dma_start(out=sb_w2, in_=w2p.ap())
            sb_w2o = singles.tile([R, 128], BF16)
            nc.sync.dma_start(out=sb_w2o, in_=w2o.ap())
            sb_w2z = singles.tile([R, 128], BF16)
            nc.sync.dma_start(out=sb_w2z, in_=w2z.ap())
            sb_bias = singles.tile([128, 18], F32)
            nc.sync.dma_start(out=sb_bias, in_=biasp.ap())
            sb_w3a = singles.tile([R, 512], BF16)
            nc.sync.dma_start(out=sb_w3a, in_=w3a.ap())
            sb_w3t2 = singles.tile([C, 512], BF16)
            nc.sync.dma_start(out=sb_w3t2, in_=w3t2.ap())
            sb_w3o = singles.tile([R, 512], BF16)
            nc.sync.dma_start(out=sb_w3o, in_=w3o.ap())
            sb_w3d = singles.tile([R, 512], BF16)
            nc.sync.dma_start(out=sb_w3d, in_=w3d.ap())
            sb_wh = singles.tile([128, 36 * 128], BF16)
            nc.sync.dma_start(out=sb_wh, in_=whp.ap())
            sb_wt = singles.tile([128, 36 * 128], BF16)
            nc.sync.dma_start(out=sb_wt, in_=wtp.ap())

            # [R, pairs, words] view: pair index u is the OUTER free dim so
            # each position's 512-word matmul moving operand is contiguous
            xv = sb_x.rearrange("p (u n) -> p u n", n=WPC)

            def xcols(row0, nrows, u):
                return xv[row0 : row0 + nrows, u, :]

            feat = []
            for j in range(6):
                fj = singles.tile([128, WPC], BF16, name=f"feat{j}")
                feat.append(fj)

            def conv_bank(name, feat_tile, bias_col, positions, n_act):
                """positions: list (per t) of lists of (lhsT_ap, rhs_ap).
                Processes positions in pairs sharing a 2-bank PSUM tile."""
                n = len(positions)
                pairs = [(i, i + 1 if i + 1 < n else None)
                         for i in range(0, n, 2)]
                # route full pairs; a trailing single always goes DVE
                full = [p for p in pairs if p[1] is not None]
                routes = dict(zip(full, _routes(len(full), n_act)))
                featw = None  # bf16 [128, 2*WPC] running max of tanh (A route)
                accw = None   # f32 [128, 2*WPC] running max pre-tanh (D route)
                featw_started = accw_started = False
                accw_wide = False
                for pr in pairs:
                    i0, i1 = pr
                    width = 2 if i1 is not None else 1
                    y2 = psum.tile([128, 2, WPC], F32, name="ypsum2", bufs=3)
                    for sl, ipos in enumerate((i0, i1)[:width]):
                        mms = positions[ipos]
                        for i, (lhsT, rhs) in enumerate(mms):
                            nc.tensor.matmul(
                                y2[:, sl, :], lhsT, rhs,
                                start=(i == 0), stop=(i == len(mms) - 1),
                            )
                    yflat = y2.rearrange("p a b -> p (a b)")[:, : width * WPC]
                    if routes.get(pr, False):
                        if not featw_started:
                            featw = singles.tile(
                                [128, 2 * WPC], BF16, name=f"fw_{name}"
                            )
                            nc.scalar.activation(
                                featw, yflat, ACTF.Tanh, bias=bias_col
                            )
                            featw_started = True
                        else:
                            scr = gscr.tile(
                                [128, 2 * WPC], BF16, name="gscr_t", bufs=4
                            )
                            nc.scalar.activation(
                                scr, yflat, ACTF.Tanh, bias=bias_col
                            )
                            nc.vector.tensor_max(featw, featw, scr)
                    else:
                        if not accw_started:
                            accw = singles.tile(
                                [128, 2 * WPC], F32, name=f"aw_{name}"
                            )
                            nc.vector.tensor_copy(accw[:, : width * WPC], yflat)
                            accw_started = True
                            accw_wide = width == 2
                        else:
                            nc.vector.tensor_max(
                                accw[:, : width * WPC],
                                accw[:, : width * WPC],
                                yflat,
                            )
                            accw_wide = accw_wide or width == 2
                # fold the two halves + merge routes into feat_tile
                amax = None  # f32 [128, WPC] pre-tanh max of the D route
                if accw_started:
                    amax = gscr.tile([128, WPC], F32, name="amax", bufs=2)
                    if accw_wide:
                        nc.vector.tensor_max(
                            amax, accw[:, 0:WPC], accw[:, WPC : 2 * WPC]
                        )
                    else:
                        nc.vector.tensor_copy(amax, accw[:, 0:WPC])
                if featw_started and accw_started:
                    nc.vector.tensor_max(
                        feat_tile, featw[:, 0:WPC], featw[:, WPC : 2 * WPC]
                    )
                    mrg = gscr.tile([128, WPC], BF16, name="mrg", bufs=2)
                    nc.scalar.activation(mrg, amax, ACTF.Tanh, bias=bias_col)
                    nc.vector.tensor_max(feat_tile, feat_tile, mrg)
                elif featw_started:
                    nc.vector.tensor_max(
                        feat_tile, featw[:, 0:WPC], featw[:, WPC : 2 * WPC]
                    )
                else:
                    nc.scalar.activation(
                        feat_tile, amax, ACTF.Tanh, bias=bias_col
                    )

            # width-1 bank -> feat[0]; position t = 2u+d reads row-block d
            pos = []
            for t in range(20):
                d, u = t % 2, t // 2
                r0 = 0 if d == 0 else D1
                pos.append([(sb_w1[r0 : r0 + C, :], xcols(r0, C, u))])
            conv_bank("w1", feat[0], sb_bias[:, 0:1], pos, ACT_PAIRS["w1"])

            # width-2 bank -> feat[1]
            pos = []
            for t in range(19):
                if t % 2 == 0:
                    u = t // 2
                    pos.append([(sb_w2[0:R, :], xcols(0, R, u))])
                else:
                    u = (t - 1) // 2
                    pos.append(
                        [
                            (sb_w2z[0:R, :], xcols(0, R, u)),
                            (sb_w2o[0:C, :], xcols(0, C, u + 1)),
                        ]
                    )
            conv_bank("w2", feat[1], sb_bias[:, 1:2], pos, ACT_PAIRS["w2"])

            # width-3 banks -> feat[2..5]
            for j in range(4):
                cs = slice(j * 128, (j + 1) * 128)
                aj = sb_w3a[:, cs]
                t2j = sb_w3t2[:, cs]
                oj = sb_w3o[:, cs]
                dj = sb_w3d[:, cs]
                pos = []
                for t in range(18):
                    if t % 2 == 0:
                        u = t // 2
                        pos.append(
                            [
                                (aj[0:R, :], xcols(0, R, u)),
                                (t2j[0:C, :], xcols(0, C, u + 1)),
                            ]
                        )
                    else:
                        u = (t - 1) // 2
                        pos.append(
                            [
                                (oj[0:R, :], xcols(0, R, u)),
                                (dj[0:R, :], xcols(0, R, u + 1)),
                            ]
                        )
                conv_bank(
                    f"w3_{j}",
                    feat[2 + j],
                    sb_bias[:, 2 + j : 3 + j],
                    pos,
                    ACT_PAIRS[f"w3_{j}"],
                )

            # highway: h = relu(Wh f + bh), t = sig(Wt f + bt),
            # out = t*(h-f) + f, all feature-major [128 out-feats, 512 words]
            for ot in range(6):
                hp2 = psum.tile([128, 2, WPC], F32, name="ypsum2", bufs=3)
                for br, wsb in ((0, sb_wh), (1, sb_wt)):
                    for kt in range(6):
                        blk = (ot * 6 + kt) * 128
                        nc.tensor.matmul(
                            hp2[:, br, :],
                            wsb[:, blk : blk + 128],
                            feat[kt],
                            start=(kt == 0),
                            stop=(kt == 5),
                        )
                h_sb = hwt.tile([128, WPC], BF16, name="h_sb", bufs=2)
                nc.scalar.activation(
                    h_sb, hp2[:, 0, :], ACTF.Relu,
                    bias=sb_bias[:, 6 + ot : 7 + ot],
                )
                t_sb = hwt.tile([128, WPC], BF16, name="t_sb", bufs=2)
                nc.scalar.activation(
                    t_sb, hp2[:, 1, :], ACTF.Sigmoid,
                    bias=sb_bias[:, 12 + ot : 13 + ot],
                )
                out_sb = hwt.tile([128, WPC], F32, name="out_sb", bufs=2)
                nc.vector.tensor_sub(h_sb, h_sb, feat[ot])
                nc.vector.tensor_mul(h_sb, t_sb, h_sb)
                nc.vector.tensor_add(out_sb, h_sb, feat[ot])
                nc.sync.dma_start(
                    out=out.ap()[ot * 128 : (ot + 1) * 128, :], in_=out_sb
                )

    nc.compile()
    return nc


def pack_inputs(ts10_input, conv_w0, conv_b0, conv_w1, conv_b1, conv_w2,
                conv_b2, wh_w, wh_b, wt_w, wt_b):
    f = np.float32
    bf = np.float16

    def padded(top, bottom, ncols):
        arr = np.zeros((R, ncols), f)
        if top is not None:
            arr[0:C] = top
        if bottom is not None:
            arr[D1:R] = bottom
        return arr.astype(bf)

    X = np.ascontiguousarray(ts10_input, dtype=f).reshape(NW, L, C)
    w1t = conv_w0[:, :, 0].T
    w1p = padded(w1t, w1t, 128)
    w2p = padded(conv_w1[:, :, 0].T, conv_w1[:, :, 1].T, 128)
    w2o = padded(conv_w1[:, :, 1].T, conv_w1[:, :, 0].T, 128)
    w2z = padded(None, conv_w1[:, :, 0].T, 128)
    w3a = padded(conv_w2[:, :, 0].T, conv_w2[:, :, 1].T, 512)
    w3t2 = np.ascontiguousarray(conv_w2[:, :, 2].T).astype(bf)
    w3o = padded(None, conv_w2[:, :, 0].T, 512)
    w3d = padded(conv_w2[:, :, 1].T, conv_w2[:, :, 2].T, 512)
    biasp = np.zeros((128, 18), f)
    biasp[:, 0] = conv_b0
    biasp[:, 1] = conv_b1
    for j in range(4):
        biasp[:, 2 + j] = conv_b2[j * 128 : (j + 1) * 128]
    for ot in range(6):
        biasp[:, 6 + ot] = wh_b[ot * 128 : (ot + 1) * 128]
        biasp[:, 12 + ot] = wt_b[ot * 128 : (ot + 1) * 128]
    whp = np.ascontiguousarray(
        wh_w.reshape(6, 128, 6, 128).transpose(3, 0, 2, 1).reshape(128, 36 * 128)
    ).astype(bf)
    wtp = np.ascontiguousarray(
        wt_w.reshape(6, 128, 6, 128).transpose(3, 0, 2, 1).reshape(128, 36 * 128)
    ).astype(bf)
    shared = dict(w1p=w1p, w2p=w2p, w2o=w2o, w2z=w2z, w3a=w3a, w3t2=w3t2,
                  w3o=w3o, w3d=w3d, biasp=biasp, whp=whp, wtp=wtp)
    in_maps = []
    for c in range(N_CORES):
        Xc = X[c * WPC : (c + 1) * WPC]            # [512, 20, 50]
        pair = Xc.reshape(WPC, U, 2, C).transpose(2, 3, 1, 0)  # [2, C, U, 512]
        xpc = np.zeros((R, WPC * U), f)
        xpc[0:C] = pair[0].reshape(C, WPC * U)
        xpc[D1:R] = pair[1].reshape(C, WPC * U)
        in_maps.append(dict(xp=xpc.astype(bf), **shared))
    return in_maps


_NC_CACHE = None


def get_nc():
    global _NC_CACHE
    if _NC_CACHE is None:
        _NC_CACHE = build_nc()
    return _NC_CACHE


def kernel(**inputs):
    in_maps = pack_inputs(**{k: np.asarray(v) for k, v in inputs.items()})
    nc = get_nc()
    res = run_bass_kernel_spmd(nc, in_maps, core_ids=list(range(N_CORES)))
    full = np.empty((NW, OUT_DIM), np.float32)
    for c in range(N_CORES):
        full[c * WPC : (c + 1) * WPC] = res.results[c]["out"].T
    return full.reshape(B, S, OUT_DIM)
